# revision 22
# baseline (speedup 1.0000x reference)
"""Trainium2 Bass kernel for nn_CausalWanModel (frame-block-causal attention).

Self-contained: hardcodes shapes from the problem spec.
  B=1, T=3120, D=1536, H=12 heads, hd=128, frame_seqlen=780 (4 frames), 8 cores.

Sharding (2D, uniform SPMD program):
  4 head-group pairs x 2 roles. Core c: pair g=c//2 owns heads 3g..3g+2;
  role r=c%2 owns one 390-token block from EACH frame (r=0: first half of
  every frame, r=1: second half). Every core's attention work is identical
  (one query block per frame; frame-f queries see (f+1)*780 keys) and the
  instruction stream is fully uniform across cores; role differences are
  carried by host-sliced inputs and 0/1 mask blends.

  Collectives (latency-minimized):
    1. ONE 8-rank AllReduce of RMSNorm sum-of-squares partials, staged with
       mask-zeroed rows so each core's contribution lands in its half-slot
       without role-dependent addressing.
    2. pairwise AllGather of rope'd UNNORMALIZED k^T and v for the 3 local
       heads — independent of the AllReduce; k's rstd is applied
       consumer-side (it is canonical across cores, no blend needed).
    3. per-head 8-rank AllToAll of normalized attention outputs; receiver
       blends pair slots with 0/1 masks, then runs the full o-projection
       for its 390 output tokens.

  Frame-causality needs no mask tensor: frame-f queries only visit key
  tiles intersecting frames <= f (tile sets are nested: f0 c f1 c f2 c f3,
  contiguous suffixes in qT) and boundary tiles get a per-partition
  additive bias column (0 / -30000) on the exp.

  Matmuls in bf16 (fp32 PSUM accumulation); statistics in fp32. Softmax
  denominators accumulate on the PE via ones-matmuls (no vector chains).
"""

import math

import numpy as np
import ml_dtypes

import concourse.bacc as bacc
import concourse.mybir as mybir
import concourse.tile as tile
from concourse.bass_utils import run_bass_kernel_spmd

F32 = mybir.dt.float32
BF16 = mybir.dt.bfloat16

NC = 8
T = 3120
D = 1536
H = 12
HD = 128
L = 780          # frame_seqlen
NFR = 4          # frames
BLK = 390        # query block (half frame)
CHUNK = BLK      # output tokens per core (test.py compat)
LT = 1560        # local tokens per core (one block per frame)
SLOT = 1664      # padded per-core token slot in the kv exchange (13*128)
KS = 2 * SLOT    # canonical key stream length (26 tiles)
NKT = KS // 128  # 26
KC = D // 128    # 12 contraction chunks
HG = 3           # heads per core
GD = HG * HD     # 384 local head dims
EPS = 1e-6
SCALE = 1.0 / math.sqrt(HD)
NEG = -30000.0

KVSZ = 2 * GD * SLOT  # k^T region [GD, SLOT] then v region [SLOT, GD]


def _stream_frames():
    """Frame id per canonical key-stream position; -1 = padding."""
    fr = []
    for _half in range(2):
        for f in range(NFR):
            fr += [f] * BLK
        fr += [-1] * (SLOT - NFR * BLK)
    return np.array(fr)


_FR = _stream_frames()


def _group_tiles():
    """Per query-frame f: [(tile, bias_col_idx|None)]; plus bias col table."""
    bias_cols = []
    groups = []
    for f in range(NFR):
        tl = []
        for kt in range(NKT):
            seg = _FR[kt * 128:(kt + 1) * 128]
            vis = (seg >= 0) & (seg <= f)
            if not vis.any():
                continue
            if vis.all():
                tl.append((kt, None))
                continue
            col = np.where(vis, 0.0, NEG).astype(np.float32)
            ci = None
            for i, c in enumerate(bias_cols):
                if np.array_equal(c, col):
                    ci = i
                    break
            if ci is None:
                bias_cols.append(col)
                ci = len(bias_cols) - 1
            tl.append((kt, ci))
        groups.append(tl)
    return groups, np.stack(bias_cols, axis=1)


GROUPS, BIASC = _group_tiles()
NB = BIASC.shape[1]
# tile -> smallest query frame that visits it (groups are nested suffixes)
TILE_FMIN = {}
TILE_BIAS = {}  # (f, kt) -> bias col idx or None
for _f in range(NFR):
    for _kt, _b in GROUPS[_f]:
        TILE_FMIN.setdefault(_kt, _f)
        TILE_BIAS[(_f, _kt)] = _b
ALL_TILES = sorted(TILE_FMIN)  # == range(NKT)


def build_kernel():
    nc = bacc.Bacc("TRN2", target_bir_lowering=False, debug=False,
                   num_devices=NC)

    # ---- I/O (per-core host-prepared) ----
    xT = nc.dram_tensor("xT", [KC, 128, LT], BF16, kind="ExternalInput")
    wq = nc.dram_tensor("wq", [KC, 128, GD], BF16, kind="ExternalInput")
    wk = nc.dram_tensor("wk", [KC, 128, GD], BF16, kind="ExternalInput")
    wv = nc.dram_tensor("wv", [KC, 128, GD], BF16, kind="ExternalInput")
    wo = nc.dram_tensor("wo", [KC, 128, D], BF16, kind="ExternalInput")
    cost = nc.dram_tensor("cost", [128, LT], F32, kind="ExternalInput")
    sint = nc.dram_tensor("sint", [128, LT], F32, kind="ExternalInput")
    selm = nc.dram_tensor("selm", [128, 2], F32, kind="ExternalInput")
    biasc = nc.dram_tensor("biasc", [128, NB], F32, kind="ExternalInput")
    gtab = nc.dram_tensor("gtab", [128, 4 * HG], F32, kind="ExternalInput")
    btab = nc.dram_tensor("btab", [128, 2 * HG], F32, kind="ExternalInput")
    bvrow = nc.dram_tensor("bvrow", [1, GD], F32, kind="ExternalInput")
    botab = nc.dram_tensor("botab", [128, KC], F32, kind="ExternalInput")
    out_part = nc.dram_tensor("out_part", [D, BLK], F32, kind="ExternalOutput")

    # ---- collective buffers ----
    # ssq layout: [qk, half, LT]; each core writes mask-zeroed rows to BOTH
    # half slots, so one 8-rank AllReduce yields the true full-D sums.
    ssq_in = nc.dram_tensor("ssq_in", [2, 2, LT], F32)
    ssq_all = nc.dram_tensor("ssq_all", [2, 2, LT], F32, addr_space="Shared")
    kv_in = nc.dram_tensor("kv_in", [KVSZ], BF16)
    kv_out = nc.dram_tensor("kv_out", [2, KVSZ], BF16)
    a2a_in = [nc.dram_tensor(f"a2a_in{h}", [NC, 128, BLK], BF16)
              for h in range(HG)]
    a2a_out = [nc.dram_tensor(f"a2a_out{h}", [NC, 128, BLK], BF16)
               for h in range(HG)]

    PAIRS = [[0, 1], [2, 3], [4, 5], [6, 7]]
    ALL8 = [list(range(NC))]

    with tile.TileContext(nc) as tc:
        with tc.tile_pool(name="const", bufs=1) as cpool:
            selm_sb = cpool.tile([128, 2], F32, tag="selm_sb")
            biasc_sb = cpool.tile([128, NB], F32, tag="biasc_sb")
            btab_sb = cpool.tile([128, 2 * HG], F32, tag="btab_sb")
            botab_sb = cpool.tile([128, KC], F32, tag="botab_sb")
            qT_sb = cpool.tile([128, HG * LT], BF16, tag="qT_sb")
            attn_all = cpool.tile([128, H, BLK], BF16, tag="attn_all")
            rk_bc = cpool.tile([128, KS], BF16, tag="rk_bc")
            ones_bf = cpool.tile([128, 1], BF16, tag="ones_bf")
            eps_sb = cpool.tile([2, 1], F32, tag="eps_sb")
            zero_bf = cpool.tile([128, GD], BF16, tag="zero_bf")

            nc.gpsimd.memset(ones_bf[:, :], 1.0)
            nc.gpsimd.memset(eps_sb[:, :], EPS)
            nc.gpsimd.memset(zero_bf[:, :], 0.0)

            nc.sync.dma_start(out=selm_sb[:, :], in_=selm[:, :])
            nc.sync.dma_start(out=biasc_sb[:, :], in_=biasc[:, :])
            nc.sync.dma_start(out=btab_sb[:, :], in_=btab[:, :])
            nc.sync.dma_start(out=botab_sb[:, :], in_=botab[:, :])

            kT_view = kv_in.ap()[0:GD * SLOT].rearrange("(d t) -> d t", t=SLOT)
            v_view = kv_in.ap()[GD * SLOT:].rearrange("(t c) -> t c", c=GD)

            # ===== Phase 1: projections + rope + ssq partials =====
            with tc.tile_pool(name="p1x", bufs=1) as p1x, \
                 tc.tile_pool(name="p1w", bufs=1) as p1w, \
                 tc.tile_pool(name="p1big", bufs=1) as p1big, \
                 tc.tile_pool(name="p1sb", bufs=4) as p1sb, \
                 tc.tile_pool(name="sqc", bufs=2) as sqcp, \
                 tc.tile_pool(name="p1ps", bufs=3, space="PSUM") as p1ps, \
                 tc.tile_pool(name="ssqps", bufs=2, space="PSUM") as ssqps:

                xT_sb = p1x.tile([128, KC * LT], BF16, tag="xT_sb")
                qrot = p1x.tile([128, HG * LT], BF16, tag="qrot")
                wq_sb = p1w.tile([128, KC * GD], BF16, tag="wq_sb")
                wk_sb = p1w.tile([128, KC * GD], BF16, tag="wk_sb")
                wv_sb = p1w.tile([128, KC * GD], BF16, tag="wv_sb")
                cost_sb = p1big.tile([128, LT], F32, tag="cost_sb")
                sint_sb = p1big.tile([128, LT], F32, tag="sint_sb")
                gtab_sb = p1big.tile([128, 4 * HG], F32, tag="gtab_sb")
                bvrow_sb = p1big.tile([1, GD], F32, tag="bvrow_sb")
                bvb = p1big.tile([128, GD], F32, tag="bvb")
                for c in range(KC):
                    nc.sync.dma_start(out=xT_sb[:, c * LT:(c + 1) * LT],
                                      in_=xT[c, :, :])
                for c in range(KC):
                    nc.sync.dma_start(out=wk_sb[:, c * GD:(c + 1) * GD],
                                      in_=wk[c, :, :])
                    nc.sync.dma_start(out=wv_sb[:, c * GD:(c + 1) * GD],
                                      in_=wv[c, :, :])
                    nc.sync.dma_start(out=wq_sb[:, c * GD:(c + 1) * GD],
                                      in_=wq[c, :, :])
                nc.sync.dma_start(out=cost_sb[:, :], in_=cost[:, :])
                nc.sync.dma_start(out=sint_sb[:, :], in_=sint[:, :])
                nc.sync.dma_start(out=gtab_sb[:, :], in_=gtab[:, :])
                nc.sync.dma_start(out=bvrow_sb[0:1, :], in_=bvrow[:, :])
                nc.gpsimd.partition_broadcast(bvb[:, :], bvrow_sb[0:1, :])

                def qk_proj(tn, w_sb, brow, grow, ssq_row, rot):
                    """Project + rope (pre-normalization) + ssq partials.

                    Loop order d-outer/c-mid/t-inner so each lhsT chunk is
                    loaded once and reused for 4 token-tile matmuls.
                    """
                    for t in range(NFR):
                        pss = [p1ps.tile([128, BLK], F32, tag=f"proj_ps{t}",
                                         name=f"proj_ps{t}")
                               for _ in range(1)]
                    for d in range(HG):
                        pst = [p1ps.tile([128, BLK], F32, tag=f"pp{t}",
                                         name=f"pp{t}") for t in range(NFR)]
                        for c in range(KC):
                            for t in range(NFR):
                                nc.tensor.matmul(
                                    pst[t][:, :],
                                    w_sb[:, c * GD + d * 128:
                                         c * GD + (d + 1) * 128],
                                    xT_sb[:, c * LT + t * BLK:
                                          c * LT + (t + 1) * BLK],
                                    start=(c == 0), stop=(c == KC - 1))
                        for t in range(NFR):
                            ps = pst[t]
                            ur = p1sb.tile([128, BLK], F32, tag="ur",
                                           name="ur")
                            nc.vector.tensor_scalar_add(
                                ur[:, :], ps[:, :],
                                btab_sb[:, brow + d:brow + d + 1])
                            sq = p1sb.tile([128, BLK], BF16, tag="sqsb",
                                           name="sqsb")
                            nc.vector.tensor_tensor(sq[:, :], ur[:, :],
                                                    ur[:, :],
                                                    mybir.AluOpType.mult)
                            nc.tensor.matmul(
                                ssq_ps[t][:, :], ones_bf[:, :], sq[:, :],
                                start=(d == 0), stop=(d == HG - 1))
                            # rope (pre-normalization)
                            usw = p1sb.tile([128, BLK], F32, tag="usw",
                                            name="usw")
                            nc.scalar.dma_start(out=usw[0:64, :],
                                                in_=ur[64:128, :])
                            nc.scalar.dma_start(out=usw[64:128, :],
                                                in_=ur[0:64, :])
                            sl = slice(t * BLK, (t + 1) * BLK)
                            t1 = p1sb.tile([128, BLK], BF16, tag="rope_t1",
                                           name="rope_t1")
                            t2 = p1sb.tile([128, BLK], BF16, tag="rope_t2",
                                           name="rope_t2")
                            nc.vector.scalar_tensor_tensor(
                                t1[:, :], ur[:, :],
                                gtab_sb[:, grow + d:grow + d + 1],
                                cost_sb[:, sl], mybir.AluOpType.mult,
                                mybir.AluOpType.mult)
                            nc.vector.scalar_tensor_tensor(
                                t2[:, :], usw[:, :],
                                gtab_sb[:, grow + HG + d:grow + HG + d + 1],
                                sint_sb[:, sl], mybir.AluOpType.mult,
                                mybir.AluOpType.mult)
                            nc.vector.tensor_tensor(
                                rot[:, d * LT + t * BLK:
                                    d * LT + (t + 1) * BLK],
                                t1[:, :], t2[:, :], mybir.AluOpType.add)
                    # ship masked ssq partials to both half slots
                    for t in range(NFR):
                        sc_sb = sqcp.tile([1, BLK], F32, tag="ssqc",
                                          name="ssqc")
                        nc.vector.tensor_copy(sc_sb[:, :], ssq_ps[t][:, :])
                        for half in range(2):
                            mrow = sqcp.tile([1, BLK], F32, tag="mrow",
                                             name="mrow")
                            nc.vector.tensor_scalar_mul(
                                mrow[:, :], sc_sb[:, :],
                                selm_sb[0:1, half:half + 1])
                            nc.scalar.dma_start(
                                out=ssq_in.ap()[ssq_row, half,
                                                t * BLK:(t + 1) * BLK],
                                in_=mrow[0, :])

                # ---- k projection + rope, then v, then kv AllGather ----
                ssq_ps = [ssqps.tile([1, BLK], F32, tag=f"ssq_ps{t}",
                                     name=f"ssq_ps{t}") for t in range(NFR)]
                krot = qT_sb  # reuse: qT not needed until after q-scale
                krot = p1x.tile([128, HG * LT], BF16, tag="krot")
                qk_proj("k", wk_sb, HG, 2 * HG, 1, krot)
                for d in range(HG):
                    for t in range(NFR):
                        sl = slice(d * LT + t * BLK, d * LT + (t + 1) * BLK)
                        kr = p1sb.tile([128, BLK], BF16, tag="krope",
                                       name="krope")
                        nc.vector.tensor_copy(kr[:, :], krot[:, sl])
                        nc.sync.dma_start(
                            out=kT_view[d * 128:(d + 1) * 128,
                                        t * BLK:(t + 1) * BLK],
                            in_=kr[:, :])
                # zero the padded kv rows/cols
                nc.scalar.dma_start(out=v_view[LT:SLOT, :],
                                    in_=zero_bf[0:SLOT - LT, :])
                for d in range(HG):
                    nc.scalar.dma_start(
                        out=kT_view[d * 128:(d + 1) * 128, LT:SLOT],
                        in_=zero_bf[:, 0:SLOT - LT])

                # ---- v projection ----
                for c in range(KC):
                    pass  # (loop below is c-outer for weight reuse)
                vps = [p1ps.tile([128, GD], F32, tag=f"v_ps{i}",
                                 name=f"v_ps{i}") for i in range(2)]
                for half13 in range(2):
                    lo = half13 * 7
                    hi = min(13, lo + 7)
                    for c in range(KC):
                        for t13 in range(lo, hi):
                            tsz = 128 if t13 < 12 else LT - 12 * 128
                            nc.tensor.matmul(
                                vps[0][0:tsz, :] if False else None, None,
                                None)
                    break

                nc.compile_abort = True  # placeholder (rewritten below)

    nc.compile()
    return nc


# revision 27
# speedup vs baseline: 1.2923x; 1.2923x over previous
"""Trainium2 Bass kernel for nn_CausalWanModel (frame-block-causal attention).

Self-contained: hardcodes shapes from the problem spec.
  B=1, T=3120, D=1536, H=12 heads, hd=128, frame_seqlen=780 (4 frames), 8 cores.

Sharding (2D, uniform SPMD program):
  4 head-group pairs x 2 roles. Core c: pair g=c//2 owns heads 3g..3g+2;
  role r=c%2 owns one 390-token block from EACH frame (r=0: first half of
  every frame, r=1: second half). Every core's attention work is identical
  (one query block per frame; frame-f queries see (f+1)*780 keys) and the
  instruction stream is fully uniform across cores; role differences are
  carried by host-sliced inputs and 0/1 mask blends.

  Collectives (latency-minimized):
    1. pairwise AllGather of rope'd UNNORMALIZED k^T and v for the 3 local
       heads — k's rstd is applied consumer-side (canonical, no blend),
       so this AllGather depends on nothing but the k/v projections.
    2. ONE 8-rank AllReduce of RMSNorm sum-of-squares partials, staged
       with mask-zeroed rows so each core's contribution lands in its
       half-slot without role-dependent addressing. Overlaps the
       AllGather; only the small q-scale multiply waits on it.
    3. per-head 8-rank AllToAll of normalized attention outputs; receiver
       blends pair slots with 0/1 masks, then runs the full o-projection
       for its 390 output tokens.

  Frame-causality needs no mask tensor: frame-f queries only visit key
  tiles intersecting frames <= f (tile sets are nested suffixes in qT)
  and boundary tiles get a per-partition additive bias column on the exp.

  Matmuls in bf16 (fp32 PSUM accumulation); statistics in fp32. Softmax
  denominators accumulate on the PE via ones-matmuls (no vector chains).
"""

import math

import numpy as np
import ml_dtypes

import concourse.bacc as bacc
import concourse.mybir as mybir
import concourse.tile as tile
from concourse.bass_utils import run_bass_kernel_spmd

F32 = mybir.dt.float32
BF16 = mybir.dt.bfloat16

NC = 8
T = 3120
D = 1536
H = 12
HD = 128
L = 780          # frame_seqlen
NFR = 4          # frames
BLK = 390        # query block (half frame)
CHUNK = BLK      # output tokens per core (test.py compat)
LT = 1560        # local tokens per core (one block per frame)
SLOT = 1664      # padded per-core token slot in the kv exchange (13*128)
KS = 2 * SLOT    # canonical key stream length (26 tiles)
NKT = KS // 128  # 26
KC = D // 128    # 12 contraction chunks
HG = 3           # heads per core
GD = HG * HD     # 384 local head dims
EPS = 1e-6
SCALE = 1.0 / math.sqrt(HD)
NEG = -30000.0

KVSZ = 2 * GD * SLOT  # k^T region [GD, SLOT] then v region [SLOT, GD]


def _stream_frames():
    """Frame id per canonical key-stream position; -1 = padding."""
    fr = []
    for _half in range(2):
        for f in range(NFR):
            fr += [f] * BLK
        fr += [-1] * (SLOT - NFR * BLK)
    return np.array(fr)


_FR = _stream_frames()


def _group_tiles():
    """Per query-frame f: [(tile, bias_col_idx|None)]; plus bias col table."""
    bias_cols = []
    groups = []
    for f in range(NFR):
        tl = []
        for kt in range(NKT):
            seg = _FR[kt * 128:(kt + 1) * 128]
            vis = (seg >= 0) & (seg <= f)
            if not vis.any():
                continue
            if vis.all():
                tl.append((kt, None))
                continue
            col = np.where(vis, 0.0, NEG).astype(np.float32)
            ci = None
            for i, c in enumerate(bias_cols):
                if np.array_equal(c, col):
                    ci = i
                    break
            if ci is None:
                bias_cols.append(col)
                ci = len(bias_cols) - 1
            tl.append((kt, ci))
        groups.append(tl)
    return groups, np.stack(bias_cols, axis=1)


GROUPS, BIASC = _group_tiles()
NB = BIASC.shape[1]
# attention waves: two query groups processed tile-outer so score lhsT
# loads amortize; PSUM budget: 2 acc + 2 sums + 2x2 sc banks = 8
WAVES = [(3, 2), (1, 0)]


def build_kernel():
    nc = bacc.Bacc("TRN2", target_bir_lowering=False, debug=False,
                   num_devices=NC)

    # ---- I/O (per-core host-prepared) ----
    xT = nc.dram_tensor("xT", [KC, 128, LT], BF16, kind="ExternalInput")
    wq = nc.dram_tensor("wq", [KC, 128, GD], BF16, kind="ExternalInput")
    wk = nc.dram_tensor("wk", [KC, 128, GD], BF16, kind="ExternalInput")
    wv = nc.dram_tensor("wv", [KC, 128, GD], BF16, kind="ExternalInput")
    wo = nc.dram_tensor("wo", [KC, 128, D], BF16, kind="ExternalInput")
    cost = nc.dram_tensor("cost", [128, LT], F32, kind="ExternalInput")
    sint = nc.dram_tensor("sint", [128, LT], F32, kind="ExternalInput")
    selm = nc.dram_tensor("selm", [128, 2], F32, kind="ExternalInput")
    biasc = nc.dram_tensor("biasc", [128, NB], F32, kind="ExternalInput")
    gtab = nc.dram_tensor("gtab", [128, 4 * HG], F32, kind="ExternalInput")
    btab = nc.dram_tensor("btab", [128, 2 * HG], F32, kind="ExternalInput")
    bvrow = nc.dram_tensor("bvrow", [1, GD], F32, kind="ExternalInput")
    botab = nc.dram_tensor("botab", [128, KC], F32, kind="ExternalInput")
    out_part = nc.dram_tensor("out_part", [D, BLK], F32, kind="ExternalOutput")

    # ---- collective buffers ----
    # ssq layout [qk, half, LT]; mask-zeroed rows -> one 8-rank AllReduce
    ssq_in = nc.dram_tensor("ssq_in", [2, 2, LT], F32)
    ssq_all = nc.dram_tensor("ssq_all", [2, 2, LT], F32, addr_space="Shared")
    kv_in = nc.dram_tensor("kv_in", [KVSZ], BF16)
    kv_out = nc.dram_tensor("kv_out", [2, KVSZ], BF16)
    a2a_in = [nc.dram_tensor(f"a2a_in{h}", [NC, 128, BLK], BF16)
              for h in range(HG)]
    a2a_out = [nc.dram_tensor(f"a2a_out{h}", [NC, 128, BLK], BF16)
               for h in range(HG)]

    PAIRS = [[0, 1], [2, 3], [4, 5], [6, 7]]
    ALL8 = [list(range(NC))]

    with tile.TileContext(nc) as tc:
        with tc.tile_pool(name="const", bufs=1) as cpool:
            selm_sb = cpool.tile([128, 2], F32, tag="selm_sb")
            biasc_sb = cpool.tile([128, NB], F32, tag="biasc_sb")
            btab_sb = cpool.tile([128, 2 * HG], F32, tag="btab_sb")
            botab_sb = cpool.tile([128, KC], F32, tag="botab_sb")
            qT_sb = cpool.tile([128, HG * LT], BF16, tag="qT_sb")
            attn_all = cpool.tile([128, H, BLK], BF16, tag="attn_all")
            rk_bc = cpool.tile([128, KS], BF16, tag="rk_bc")
            ones_bf = cpool.tile([128, 1], BF16, tag="ones_bf")
            eps_sb = cpool.tile([2, 1], F32, tag="eps_sb")
            zero_bf = cpool.tile([128, GD], BF16, tag="zero_bf")

            nc.gpsimd.memset(ones_bf[:, :], 1.0)
            nc.gpsimd.memset(eps_sb[:, :], EPS)
            nc.gpsimd.memset(zero_bf[:, :], 0.0)

            nc.sync.dma_start(out=selm_sb[:, :], in_=selm[:, :])
            nc.sync.dma_start(out=biasc_sb[:, :], in_=biasc[:, :])
            nc.sync.dma_start(out=btab_sb[:, :], in_=btab[:, :])
            nc.sync.dma_start(out=botab_sb[:, :], in_=botab[:, :])

            kT_view = kv_in.ap()[0:GD * SLOT].rearrange("(d t) -> d t", t=SLOT)
            v_view = kv_in.ap()[GD * SLOT:].rearrange("(t c) -> t c", c=GD)

            # ===== Phase 1: projections + rope + ssq partials =====
            with tc.tile_pool(name="p1x", bufs=1) as p1x, \
                 tc.tile_pool(name="p1w", bufs=1) as p1w, \
                 tc.tile_pool(name="p1big", bufs=1) as p1big, \
                 tc.tile_pool(name="p1sb", bufs=3) as p1sb, \
                 tc.tile_pool(name="sqc", bufs=2) as sqcp, \
                 tc.tile_pool(name="p1ps", bufs=2, space="PSUM") as p1ps, \
                 tc.tile_pool(name="ssqps", bufs=1, space="PSUM") as ssqps:

                xT_sb = p1x.tile([128, KC * LT], BF16, tag="xT_sb")
                qrot = p1x.tile([128, HG * LT], BF16, tag="qrot")
                krot = p1x.tile([128, HG * LT], BF16, tag="krot")
                wq_sb = p1w.tile([128, KC * GD], BF16, tag="wq_sb")
                wk_sb = p1w.tile([128, KC * GD], BF16, tag="wk_sb")
                wv_sb = p1w.tile([128, KC * GD], BF16, tag="wv_sb")
                cost_sb = p1big.tile([128, LT], F32, tag="cost_sb")
                sint_sb = p1big.tile([128, LT], F32, tag="sint_sb")
                gtab_sb = p1big.tile([128, 4 * HG], F32, tag="gtab_sb")
                bvrow_sb = p1big.tile([1, GD], F32, tag="bvrow_sb")
                bvb = p1big.tile([128, GD], F32, tag="bvb")
                for c in range(KC):
                    nc.sync.dma_start(out=xT_sb[:, c * LT:(c + 1) * LT],
                                      in_=xT[c, :, :])
                for c in range(KC):
                    nc.sync.dma_start(out=wk_sb[:, c * GD:(c + 1) * GD],
                                      in_=wk[c, :, :])
                    nc.sync.dma_start(out=wv_sb[:, c * GD:(c + 1) * GD],
                                      in_=wv[c, :, :])
                    nc.sync.dma_start(out=wq_sb[:, c * GD:(c + 1) * GD],
                                      in_=wq[c, :, :])
                nc.sync.dma_start(out=cost_sb[:, :], in_=cost[:, :])
                nc.sync.dma_start(out=sint_sb[:, :], in_=sint[:, :])
                nc.sync.dma_start(out=gtab_sb[:, :], in_=gtab[:, :])
                nc.sync.dma_start(out=bvrow_sb[0:1, :], in_=bvrow[:, :])
                nc.gpsimd.partition_broadcast(bvb[:, :], bvrow_sb[0:1, :])

                def qk_proj(tn, w_sb, brow, grow, ssq_row, rot):
                    """Project + rope (pre-norm) + ssq partials.

                    d-outer / t-pair / c-inner: each lhsT chunk loads once
                    per token pair (2 matmuls per LDWEIGHTS).
                    """
                    ssq_ps = [ssqps.tile([1, BLK], F32, tag=f"ssq{t}",
                                         name=f"ssq_{tn}{t}")
                              for t in range(NFR)]
                    for d in range(HG):
                        for tp in range(2):
                            ts = (2 * tp, 2 * tp + 1)
                            pst = {t: p1ps.tile([128, BLK], F32,
                                                tag=f"pp{t % 2}",
                                                name=f"pp{t % 2}")
                                   for t in ts}
                            for c in range(KC):
                                for t in ts:
                                    nc.tensor.matmul(
                                        pst[t][:, :],
                                        w_sb[:, c * GD + d * 128:
                                             c * GD + (d + 1) * 128],
                                        xT_sb[:, c * LT + t * BLK:
                                              c * LT + (t + 1) * BLK],
                                        start=(c == 0), stop=(c == KC - 1))
                            for t in ts:
                                ps = pst[t]
                                ur = p1sb.tile([128, BLK], F32, tag="ur",
                                               name="ur")
                                nc.vector.tensor_scalar_add(
                                    ur[:, :], ps[:, :],
                                    btab_sb[:, brow + d:brow + d + 1])
                                sq = p1sb.tile([128, BLK], BF16, tag="sqsb",
                                               name="sqsb")
                                nc.vector.tensor_tensor(
                                    sq[:, :], ur[:, :], ur[:, :],
                                    mybir.AluOpType.mult)
                                nc.tensor.matmul(
                                    ssq_ps[t][:, :], ones_bf[:, :], sq[:, :],
                                    start=(d == 0), stop=(d == HG - 1))
                                # rope (pre-normalization)
                                usw = p1sb.tile([128, BLK], F32, tag="usw",
                                                name="usw")
                                nc.scalar.dma_start(out=usw[0:64, :],
                                                    in_=ur[64:128, :])
                                nc.scalar.dma_start(out=usw[64:128, :],
                                                    in_=ur[0:64, :])
                                sl = slice(t * BLK, (t + 1) * BLK)
                                t1 = p1sb.tile([128, BLK], BF16,
                                               tag="rope_t1", name="rope_t1")
                                t2 = p1sb.tile([128, BLK], BF16,
                                               tag="rope_t2", name="rope_t2")
                                nc.vector.scalar_tensor_tensor(
                                    t1[:, :], ur[:, :],
                                    gtab_sb[:, grow + d:grow + d + 1],
                                    cost_sb[:, sl], mybir.AluOpType.mult,
                                    mybir.AluOpType.mult)
                                nc.vector.scalar_tensor_tensor(
                                    t2[:, :], usw[:, :],
                                    gtab_sb[:, grow + HG + d:
                                            grow + HG + d + 1],
                                    sint_sb[:, sl], mybir.AluOpType.mult,
                                    mybir.AluOpType.mult)
                                nc.vector.tensor_tensor(
                                    rot[:, d * LT + t * BLK:
                                        d * LT + (t + 1) * BLK],
                                    t1[:, :], t2[:, :], mybir.AluOpType.add)
                    # ship mask-zeroed ssq partials to both half slots
                    for t in range(NFR):
                        sc_sb = sqcp.tile([1, BLK], F32, tag="ssqc",
                                          name="ssqc")
                        nc.vector.tensor_copy(sc_sb[:, :], ssq_ps[t][:, :])
                        for half in range(2):
                            mrow = sqcp.tile([1, BLK], F32, tag="mrow",
                                             name="mrow")
                            nc.vector.tensor_scalar_mul(
                                mrow[:, :], sc_sb[:, :],
                                selm_sb[0:1, half:half + 1])
                            nc.scalar.dma_start(
                                out=ssq_in.ap()[ssq_row, half,
                                                t * BLK:(t + 1) * BLK],
                                in_=mrow[0:1, :])

                # ---- k proj+rope -> stage unnormalized k ----
                qk_proj("k", wk_sb, HG, 2 * HG, 1, krot)
                for d in range(HG):
                    for t in range(NFR):
                        nc.sync.dma_start(
                            out=kT_view[d * 128:(d + 1) * 128,
                                        t * BLK:(t + 1) * BLK],
                            in_=krot[:, d * LT + t * BLK:
                                     d * LT + (t + 1) * BLK])
                nc.scalar.dma_start(out=v_view[LT:SLOT, :],
                                    in_=zero_bf[0:SLOT - LT, :])
                for d in range(HG):
                    nc.scalar.dma_start(
                        out=kT_view[d * 128:(d + 1) * 128, LT:SLOT],
                        in_=zero_bf[:, 0:SLOT - LT])

                # ---- v projection (t13-pairs outer, c inner: 2 MM/LDW) ----
                for vp in range(7):
                    ts = [t for t in (2 * vp, 2 * vp + 1) if t < 13]
                    vps = {t: p1ps.tile([128, BLK], F32, tag=f"pp{t % 2}",
                                        name=f"vp{t % 2}") for t in ts}
                    for c in range(KC):
                        for t in ts:
                            tsz = 128 if t < 12 else LT - 12 * 128
                            nc.tensor.matmul(
                                vps[t][0:tsz, 0:GD],
                                xT_sb[:, c * LT + t * 128:
                                      c * LT + t * 128 + tsz],
                                wv_sb[:, c * GD:(c + 1) * GD],
                                start=(c == 0), stop=(c == KC - 1))
                    for t in ts:
                        tsz = 128 if t < 12 else LT - 12 * 128
                        vsb = p1sb.tile([128, GD], BF16, tag="vsb",
                                        name="vsb")
                        nc.vector.tensor_tensor(vsb[0:tsz, :],
                                                vps[t][0:tsz, 0:GD],
                                                bvb[0:tsz, :],
                                                mybir.AluOpType.add)
                        nc.sync.dma_start(
                            out=v_view[t * 128:t * 128 + tsz, :],
                            in_=vsb[0:tsz, :])

                # kv exchange: independent of the rmsnorm AllReduce
                nc.gpsimd.collective_compute(
                    "AllGather", mybir.AluOpType.bypass,
                    ins=[kv_in.ap().opt()], outs=[kv_out.ap().opt()],
                    replica_groups=PAIRS)

                # ---- q proj+rope (overlaps the AllGather) ----
                qk_proj("q", wq_sb, 0, 0, 0, qrot)
                nc.gpsimd.collective_compute(
                    "AllReduce", mybir.AluOpType.add,
                    ins=[ssq_in.ap().opt()], outs=[ssq_all.ap().opt()],
                    replica_groups=ALL8)

                # ---- rstd tables from the AllReduce ----
                # q: blend halves with my role mask, broadcast, scale qrot
                sa_q = p1big.tile([2, LT], F32, tag="sa_q")
                nc.sync.dma_start(out=sa_q[:, :], in_=ssq_all[0, :, :])
                rst_q = p1big.tile([2, LT], F32, tag="rst_q")
                nc.scalar.activation(rst_q[:, :], sa_q[:, :],
                                     mybir.ActivationFunctionType.Sqrt,
                                     bias=eps_sb[:, :], scale=1.0 / D)
                nc.vector.reciprocal_approx_fast(sa_q[:, :], rst_q[:, :])
                rowb = p1big.tile([1, LT], F32, tag="rowb")
                nc.scalar.dma_start(out=rowb[0:1, :], in_=sa_q[1:2, :])
                rowt = p1big.tile([1, LT], F32, tag="rowt")
                nc.vector.tensor_scalar_mul(rowt[:, :], sa_q[0:1, :],
                                            selm_sb[0:1, 0:1])
                rowf = p1big.tile([1, LT], F32, tag="rowf")
                nc.vector.scalar_tensor_tensor(
                    rowf[:, :], rowb[:, :], selm_sb[0:1, 1:2], rowt[:, :],
                    mybir.AluOpType.mult, mybir.AluOpType.add)
                bcq = p1big.tile([128, LT], F32, tag="bcq")
                nc.gpsimd.partition_broadcast(bcq[:, :], rowf[:, :])
                for d in range(HG):
                    for t in range(NFR):
                        sl = slice(d * LT + t * BLK, d * LT + (t + 1) * BLK)
                        nc.vector.tensor_tensor(
                            qT_sb[:, sl], qrot[:, sl],
                            bcq[:, t * BLK:(t + 1) * BLK],
                            mybir.AluOpType.mult)

                # k: canonical (no blend) -> padded bf16 stream row -> bcast
                sa_k = p1big.tile([2, LT], F32, tag="sa_q", name="sa_k")
                nc.sync.dma_start(out=sa_k[:, :], in_=ssq_all[1, :, :])
                rst_k = p1big.tile([2, LT], F32, tag="rst_q", name="rst_k")
                nc.scalar.activation(rst_k[:, :], sa_k[:, :],
                                     mybir.ActivationFunctionType.Sqrt,
                                     bias=eps_sb[:, :], scale=1.0 / D)
                nc.vector.reciprocal_approx_fast(sa_k[:, :], rst_k[:, :])
                rowk1 = p1big.tile([1, LT], F32, tag="rowb", name="rowk1")
                nc.scalar.dma_start(out=rowk1[0:1, :], in_=sa_k[1:2, :])
                rk_pad = p1big.tile([1, KS], BF16, tag="rk_pad")
                nc.vector.memset(rk_pad[:, :], 0.0)
                nc.vector.tensor_copy(rk_pad[0:1, 0:LT], sa_k[0:1, :])
                nc.vector.tensor_copy(rk_pad[0:1, SLOT:SLOT + LT],
                                      rowk1[0:1, :])
                nc.gpsimd.partition_broadcast(rk_bc[:, :], rk_pad[0:1, :])

            # ===== Phase 2: attention (3 local heads) =====
            with tc.tile_pool(name="p4w", bufs=1) as p4w:
                wo_sb = p4w.tile([128, KC * D], BF16, tag="wo_sb")
                for c in range(KC):
                    nc.sync.dma_start(out=wo_sb[:, c * D:(c + 1) * D],
                                      in_=wo[c, :, :])

                with tc.tile_pool(name="a_k", bufs=2) as akp, \
                     tc.tile_pool(name="a_v", bufs=2) as avp, \
                     tc.tile_pool(name="a_p", bufs=6) as app, \
                     tc.tile_pool(name="a_sb", bufs=4) as asb, \
                     tc.tile_pool(name="a_ps", bufs=1, space="PSUM") as aps, \
                     tc.tile_pool(name="acc_ps", bufs=1, space="PSUM") as accps, \
                     tc.tile_pool(name="sum_ps", bufs=1, space="PSUM") as sumps:

                    for h in range(HG):
                        ktr = akp.tile([128, KS], BF16, tag="ktr", name="ktr")
                        ktn = akp.tile([128, KS], BF16, tag="ktn", name="ktn")
                        vt_sb = avp.tile([128, NKT, 128], BF16, tag="vt_sb",
                                         name="vt_sb")
                        for s in range(2):
                            nc.sync.dma_start(
                                out=ktr[:, s * SLOT:(s + 1) * SLOT],
                                in_=kv_out.ap()[s, 0:GD * SLOT]
                                .rearrange("(d t) -> d t", t=SLOT)
                                [h * 128:(h + 1) * 128, :])
                            nc.sync.dma_start(
                                out=vt_sb[:, s * 13:(s + 1) * 13, :],
                                in_=kv_out.ap()[s, GD * SLOT:]
                                .rearrange("(t c) -> t c", c=GD)
                                [:, h * 128:(h + 1) * 128]
                                .rearrange("(tt p) c -> p tt c", p=128))
                        # consumer-side k normalization (canonical rstd)
                        nc.vector.tensor_tensor(ktn[:, :], ktr[:, :],
                                                rk_bc[:, :],
                                                mybir.AluOpType.mult)

                        for wave in WAVES:
                            accs, sums, state = {}, {}, {}
                            for f in wave:
                                accs[f] = accps.tile([128, BLK], F32,
                                                     tag=f"acc{f % 2}",
                                                     name=f"acc{f % 2}")
                                sums[f] = sumps.tile([1, BLK], F32,
                                                     tag=f"sum{f % 2}",
                                                     name=f"sum{f % 2}")
                                tl = GROUPS[f]
                                state[f] = {
                                    "tiles": dict(tl), "n": len(tl),
                                    "done": 0, "pend": [], "sc": None,
                                }

                            def flush(f, h=h):
                                st = state[f]
                                sc, pend = st["sc"], st["pend"]
                                nb = len(pend)
                                if nb == 0:
                                    return
                                pr = app.tile([128, 2, BLK], BF16,
                                              tag=f"pr{f % 2}",
                                              name=f"pr{f % 2}")
                                bci = st["tiles"][pend[0]]
                                bias = (0.0 if bci is None
                                        else biasc_sb[:, bci:bci + 1])
                                nc.scalar.activation(
                                    pr[:, 0:nb, :], sc[:, 0:nb, 0:BLK],
                                    mybir.ActivationFunctionType.Exp,
                                    bias=bias, scale=SCALE)
                                for j, kt in enumerate(pend):
                                    first = st["done"] == 0
                                    last = st["done"] == st["n"] - 1
                                    nc.tensor.matmul(
                                        accs[f][:, :], vt_sb[:, kt, :],
                                        pr[:, j, :], start=first, stop=last)
                                    nc.tensor.matmul(
                                        sums[f][:, :], ones_bf[:, :],
                                        pr[:, j, :], start=first, stop=last)
                                    st["done"] += 1
                                st["pend"] = []
                                st["sc"] = None

                            for kt in range(NKT):
                                for f in wave:
                                    st = state[f]
                                    if kt not in st["tiles"]:
                                        continue
                                    bci = st["tiles"][kt]
                                    if st["pend"]:
                                        same = (st["tiles"][st["pend"][0]]
                                                is None) and (bci is None)
                                        if not same:
                                            flush(f)
                                    if not st["pend"]:
                                        st["sc"] = aps.tile(
                                            [128, 2, 512], F32,
                                            tag=f"sc{f % 2}",
                                            name=f"sc{f % 2}")
                                    j = len(st["pend"])
                                    nc.tensor.matmul(
                                        st["sc"][:, j, 0:BLK],
                                        ktn[:, kt * 128:(kt + 1) * 128],
                                        qT_sb[:, h * LT + f * BLK:
                                              h * LT + (f + 1) * BLK],
                                        start=True, stop=True)
                                    st["pend"].append(kt)
                                    if len(st["pend"]) == 2 or bci is not None:
                                        flush(f)
                            for f in wave:
                                flush(f)
                                rec = asb.tile([1, BLK], F32,
                                               tag="rec", name="rec")
                                nc.vector.reciprocal_approx_fast(
                                    rec[:, :], sums[f][:, :])
                                recb = asb.tile([128, BLK], F32, tag="recb",
                                                name="recb")
                                nc.gpsimd.partition_broadcast(recb[:, :],
                                                              rec[:, :])
                                ab = asb.tile([128, BLK], BF16, tag="ab",
                                              name="ab")
                                nc.vector.tensor_tensor(
                                    ab[:, :], accs[f][:, :], recb[:, :],
                                    mybir.AluOpType.mult)
                                nc.scalar.dma_start(
                                    out=a2a_in[h].ap()[2 * f, :, :],
                                    in_=ab[:, :])
                                nc.scalar.dma_start(
                                    out=a2a_in[h].ap()[2 * f + 1, :, :],
                                    in_=ab[:, :])
                        nc.gpsimd.collective_compute(
                            "AllToAll", mybir.AluOpType.bypass,
                            ins=[a2a_in[h].ap().opt()],
                            outs=[a2a_out[h].ap().opt()],
                            replica_groups=ALL8)

                    # ---- blend A2A slots into attn_all ----
                    for h in range(HG):
                        for g in range(4):
                            raw = asb.tile([128, 2, BLK], BF16, tag="raw",
                                           name="raw")
                            nc.sync.dma_start(
                                out=raw[:, :, :],
                                in_=a2a_out[h].ap()[2 * g:2 * g + 2, :, :]
                                .rearrange("s p b -> p s b"))
                            tmpb = asb.tile([128, BLK], BF16, tag="tmpb",
                                            name="tmpb")
                            nc.vector.tensor_scalar_mul(
                                tmpb[:, :], raw[:, 0, :], selm_sb[:, 0:1])
                            nc.vector.scalar_tensor_tensor(
                                attn_all[:, 3 * g + h, :], raw[:, 1, :],
                                selm_sb[:, 1:2], tmpb[:, :],
                                mybir.AluOpType.mult, mybir.AluOpType.add)

                # ===== Phase 3: o-projection (out^T form, my 390 tokens) ====
                with tc.tile_pool(name="p5sb", bufs=3) as p5sb, \
                     tc.tile_pool(name="p5ps", bufs=3, space="PSUM") as p5ps:
                    for dch in range(KC):
                        ps = p5ps.tile([128, BLK], F32, tag="o_ps",
                                       name="o_ps")
                        for c in range(KC):
                            nc.tensor.matmul(
                                ps[:, :],
                                wo_sb[:, c * D + dch * 128:
                                      c * D + (dch + 1) * 128],
                                attn_all[:, c, :],
                                start=(c == 0), stop=(c == KC - 1))
                        osb = p5sb.tile([128, BLK], F32, tag="osb",
                                        name="osb")
                        nc.vector.tensor_scalar_add(osb[:, :], ps[:, :],
                                                    botab_sb[:, dch:dch + 1])
                        nc.sync.dma_start(
                            out=out_part[dch * 128:(dch + 1) * 128, :],
                            in_=osb[:, :])

    nc.compile()
    return nc


_NC_CACHE = {}


def _get_nc(key=()):
    if key not in _NC_CACHE:
        _NC_CACHE[key] = build_kernel()
    return _NC_CACHE[key]


def _prep_inputs(x, freqs_cos, freqs_sin, Wq, bq, Wk, bk, Wv, bv, Wo, bo,
                 gq, gk, frame_seqlen, debug=False):
    assert int(frame_seqlen) == L
    bf16 = ml_dtypes.bfloat16
    x2d = np.asarray(x, np.float32).reshape(T, D)
    xT_full = np.ascontiguousarray(x2d.T)

    # rope pair permutation (even dims then odd dims within each head)
    perm = np.concatenate([
        np.concatenate([np.arange(0, 128, 2), np.arange(1, 128, 2)]) + 128 * h
        for h in range(H)])
    Wqp = np.asarray(Wq, np.float32)[:, perm]
    Wkp = np.asarray(Wk, np.float32)[:, perm]
    bqp = np.asarray(bq, np.float32)[perm]
    bkp = np.asarray(bk, np.float32)[perm]
    gqp = np.asarray(gq, np.float32)[perm]
    gkp = np.asarray(gk, np.float32)[perm]
    Wv_ = np.asarray(Wv, np.float32)
    Wo_ = np.asarray(Wo, np.float32)
    bv_ = np.asarray(bv, np.float32)
    bo_ = np.asarray(bo, np.float32)

    cosT = np.asarray(freqs_cos, np.float32).T  # [64, T]
    sinT = np.asarray(freqs_sin, np.float32).T
    costab = np.concatenate([cosT, cosT], 0)     # [128, T]
    sintab = np.concatenate([-sinT, sinT], 0)

    wo_tiled = np.ascontiguousarray(Wo_.reshape(KC, 128, D)).astype(bf16)
    botab = np.ascontiguousarray(bo_.reshape(KC, 128).T)

    in_maps = []
    for c in range(NC):
        g, role = c // 2, c % 2
        tok = np.concatenate([np.arange(f * L + role * BLK,
                                        f * L + role * BLK + BLK)
                              for f in range(NFR)])
        gsl = slice(g * GD, (g + 1) * GD)

        def lhsT_chunks(w):  # [D, GD] -> [KC, 128, GD]
            return np.ascontiguousarray(w.reshape(KC, 128, GD)).astype(bf16)

        def swap_half(col):  # [128] -> halves swapped
            return np.concatenate([col[64:], col[:64]])

        gq_cols = [gqp[gsl][d * 128:(d + 1) * 128] for d in range(HG)]
        gk_cols = [gkp[gsl][d * 128:(d + 1) * 128] for d in range(HG)]
        gtab = np.stack(
            gq_cols + [swap_half(cc) for cc in gq_cols] +
            gk_cols + [swap_half(cc) for cc in gk_cols], axis=1)
        btab = np.stack(
            [bqp[gsl][d * 128:(d + 1) * 128] for d in range(HG)] +
            [bkp[gsl][d * 128:(d + 1) * 128] for d in range(HG)], axis=1)
        selm = np.zeros((128, 2), np.float32)
        selm[:, 0] = 1.0 - role
        selm[:, 1] = role

        in_maps.append({
            "xT": np.ascontiguousarray(
                xT_full[:, tok].reshape(KC, 128, LT)).astype(bf16),
            "wq": lhsT_chunks(Wqp[:, gsl]),
            "wk": lhsT_chunks(Wkp[:, gsl]),
            "wv": lhsT_chunks(Wv_[:, gsl]),
            "wo": wo_tiled,
            "cost": np.ascontiguousarray(costab[:, tok]),
            "sint": np.ascontiguousarray(sintab[:, tok]),
            "selm": selm,
            "biasc": BIASC,
            "gtab": np.ascontiguousarray(gtab.astype(np.float32)),
            "btab": np.ascontiguousarray(btab.astype(np.float32)),
            "bvrow": np.ascontiguousarray(bv_[gsl][None, :]),
            "botab": botab,
        })
    return (), in_maps


def kernel(x, freqs_cos, freqs_sin, Wq, bq, Wk, bk, Wv, bv, Wo, bo,
           gq, gk, frame_seqlen):
    key, in_maps = _prep_inputs(x, freqs_cos, freqs_sin, Wq, bq, Wk, bk,
                                Wv, bv, Wo, bo, gq, gk, frame_seqlen)
    nc = _get_nc(key)
    res = run_bass_kernel_spmd(nc, in_maps, core_ids=list(range(NC)))
    out = np.empty((1, T, D), np.float32)
    for c in range(NC):
        out[0, c * BLK:(c + 1) * BLK, :] = res.results[c]["out_part"].T
    return out


# revision 31
# speedup vs baseline: 1.6933x; 1.3103x over previous
"""Trainium2 Bass kernel for nn_CausalWanModel (frame-block-causal attention).

Self-contained: hardcodes shapes from the problem spec.
  B=1, T=3120, D=1536, H=12 heads, hd=128, frame_seqlen=780 (4 frames), 8 cores.

Sharding (2D, uniform SPMD program):
  4 head-group pairs x 2 roles. Core c: pair g=c//2 owns heads 3g..3g+2;
  role r=c%2 owns one 390-token block from EACH frame (r=0: first half of
  every frame, r=1: second half). Every core's attention work is identical
  (one query block per frame; frame-f queries see (f+1)*780 keys) and the
  instruction stream is fully uniform across cores; role differences are
  carried by host-sliced inputs and 0/1 mask blends.

  Collectives (latency-minimized):
    1. pairwise AllGather of rope'd UNNORMALIZED k^T and v for the 3 local
       heads — k's rstd is applied consumer-side (canonical, no blend),
       so this AllGather depends on nothing but the k/v projections.
    2. ONE 8-rank AllReduce of RMSNorm sum-of-squares partials, staged
       with mask-zeroed rows so each core's contribution lands in its
       half-slot without role-dependent addressing. Overlaps the
       AllGather; only the small q-scale multiply waits on it.
    3. per-head 8-rank AllToAll of normalized attention outputs; receiver
       blends pair slots with 0/1 masks, then runs the full o-projection
       for its 390 output tokens.

  Frame-causality needs no mask tensor: frame-f queries only visit key
  tiles intersecting frames <= f (tile sets are nested suffixes in qT)
  and boundary tiles get a per-partition additive bias column on the exp.

  Matmuls in bf16 (fp32 PSUM accumulation); statistics in fp32. Softmax
  denominators accumulate on the PE via ones-matmuls (no vector chains).
"""

import math

import numpy as np
import ml_dtypes

import concourse.bacc as bacc
import concourse.mybir as mybir
import concourse.tile as tile
from concourse.bass_utils import run_bass_kernel_spmd

F32 = mybir.dt.float32
BF16 = mybir.dt.bfloat16

NC = 8
T = 3120
D = 1536
H = 12
HD = 128
L = 780          # frame_seqlen
NFR = 4          # frames
BLK = 390        # query block (half frame)
CHUNK = BLK      # output tokens per core (test.py compat)
LT = 1560        # local tokens per core (one block per frame)
SLOT = 1664      # padded per-core token slot in the kv exchange (13*128)
KS = 2 * SLOT    # canonical key stream length (26 tiles)
NKT = KS // 128  # 26
KC = D // 128    # 12 contraction chunks
HG = 3           # heads per core
GD = HG * HD     # 384 local head dims
EPS = 1e-6
SCALE = 1.0 / math.sqrt(HD)
NEG = -30000.0

KVSZ = 2 * GD * SLOT  # k^T region [GD, SLOT] then v region [SLOT, GD]


def _stream_frames():
    """Frame id per canonical key-stream position; -1 = padding."""
    fr = []
    for _half in range(2):
        for f in range(NFR):
            fr += [f] * BLK
        fr += [-1] * (SLOT - NFR * BLK)
    return np.array(fr)


_FR = _stream_frames()


def _group_tiles():
    """Per query-frame f: [(tile, bias_col_idx|None)]; plus bias col table."""
    bias_cols = []
    groups = []
    for f in range(NFR):
        tl = []
        for kt in range(NKT):
            seg = _FR[kt * 128:(kt + 1) * 128]
            vis = (seg >= 0) & (seg <= f)
            if not vis.any():
                continue
            if vis.all():
                tl.append((kt, None))
                continue
            col = np.where(vis, 0.0, NEG).astype(np.float32)
            ci = None
            for i, c in enumerate(bias_cols):
                if np.array_equal(c, col):
                    ci = i
                    break
            if ci is None:
                bias_cols.append(col)
                ci = len(bias_cols) - 1
            tl.append((kt, ci))
        groups.append(tl)
    return groups, np.stack(bias_cols, axis=1)


GROUPS, BIASC = _group_tiles()
NB = BIASC.shape[1]
# attention waves: two query groups processed tile-outer so score lhsT
# loads amortize; PSUM budget: 2 acc + 2 sums + 2x2 sc banks = 8
WAVES = [(3, 2), (1, 0)]


def build_kernel():
    nc = bacc.Bacc("TRN2", target_bir_lowering=False, debug=False,
                   num_devices=NC)

    # ---- I/O (per-core host-prepared) ----
    xT = nc.dram_tensor("xT", [KC, 128, LT], BF16, kind="ExternalInput")
    wq = nc.dram_tensor("wq", [KC, 128, GD], BF16, kind="ExternalInput")
    wk = nc.dram_tensor("wk", [KC, 128, GD], BF16, kind="ExternalInput")
    wv = nc.dram_tensor("wv", [KC, 128, GD], BF16, kind="ExternalInput")
    wo = nc.dram_tensor("wo", [KC, 128, D], BF16, kind="ExternalInput")
    cost = nc.dram_tensor("cost", [128, LT], F32, kind="ExternalInput")
    sint = nc.dram_tensor("sint", [128, LT], F32, kind="ExternalInput")
    selm = nc.dram_tensor("selm", [128, 2], F32, kind="ExternalInput")
    biasc = nc.dram_tensor("biasc", [128, NB], F32, kind="ExternalInput")
    gtab = nc.dram_tensor("gtab", [128, 4 * HG], F32, kind="ExternalInput")
    btab = nc.dram_tensor("btab", [128, 2 * HG], F32, kind="ExternalInput")
    bvrow = nc.dram_tensor("bvrow", [1, GD], F32, kind="ExternalInput")
    botab = nc.dram_tensor("botab", [128, KC], F32, kind="ExternalInput")
    out_part = nc.dram_tensor("out_part", [D, BLK], F32, kind="ExternalOutput")

    # ---- collective buffers ----
    # ssq layout [qk, half, LT]; mask-zeroed rows -> one 8-rank AllReduce
    ssq_in = nc.dram_tensor("ssq_in", [2, 2, LT], F32)
    ssq_all = nc.dram_tensor("ssq_all", [2, 2, LT], F32, addr_space="Shared")
    k_in = nc.dram_tensor("k_in", [GD * SLOT], BF16)
    k_out = nc.dram_tensor("k_out", [2, GD * SLOT], BF16)
    v_in = nc.dram_tensor("v_in", [SLOT * GD], BF16)
    v_out = nc.dram_tensor("v_out", [2, SLOT * GD], BF16)
    a2a_in = [nc.dram_tensor(f"a2a_in{h}", [NC, 128, BLK], BF16)
              for h in range(HG)]
    a2a_out = [nc.dram_tensor(f"a2a_out{h}", [NC, 128, BLK], BF16)
               for h in range(HG)]

    PAIRS = [[0, 1], [2, 3], [4, 5], [6, 7]]
    ALL8 = [list(range(NC))]

    with tile.TileContext(nc) as tc:
        with tc.tile_pool(name="const", bufs=1) as cpool:
            selm_sb = cpool.tile([128, 2], F32, tag="selm_sb")
            biasc_sb = cpool.tile([128, NB], F32, tag="biasc_sb")
            btab_sb = cpool.tile([128, 2 * HG], F32, tag="btab_sb")
            botab_sb = cpool.tile([128, KC], F32, tag="botab_sb")
            qT_sb = cpool.tile([128, HG * LT], BF16, tag="qT_sb")
            attn_all = cpool.tile([128, H, BLK], BF16, tag="attn_all")
            rk_bc = cpool.tile([128, KS], BF16, tag="rk_bc")
            ones_bf = cpool.tile([128, 1], BF16, tag="ones_bf")
            eps_sb = cpool.tile([2, 1], F32, tag="eps_sb")
            zero_bf = cpool.tile([128, GD], BF16, tag="zero_bf")

            nc.gpsimd.memset(ones_bf[:, :], 1.0)
            nc.gpsimd.memset(eps_sb[:, :], EPS)
            nc.gpsimd.memset(zero_bf[:, :], 0.0)

            nc.sync.dma_start(out=selm_sb[:, :], in_=selm[:, :])
            nc.sync.dma_start(out=biasc_sb[:, :], in_=biasc[:, :])
            nc.sync.dma_start(out=btab_sb[:, :], in_=btab[:, :])
            nc.sync.dma_start(out=botab_sb[:, :], in_=botab[:, :])

            kT_view = k_in.ap().rearrange("(d t) -> d t", t=SLOT)
            v_view = v_in.ap().rearrange("(t c) -> t c", c=GD)

            # ===== Phase 1: projections + rope + ssq partials =====
            with tc.tile_pool(name="p1x", bufs=1) as p1x, \
                 tc.tile_pool(name="p1w", bufs=1) as p1w, \
                 tc.tile_pool(name="p1big", bufs=1) as p1big, \
                 tc.tile_pool(name="p1sb", bufs=3) as p1sb, \
                 tc.tile_pool(name="sqc", bufs=2) as sqcp, \
                 tc.tile_pool(name="p1ps", bufs=2, space="PSUM") as p1ps, \
                 tc.tile_pool(name="ssqps", bufs=1, space="PSUM") as ssqps:

                xT_sb = p1x.tile([128, KC * LT], BF16, tag="xT_sb")
                qrot = p1x.tile([128, HG * LT], BF16, tag="qrot")
                krot = p1x.tile([128, HG * LT], BF16, tag="krot")
                wq_sb = p1w.tile([128, KC * GD], BF16, tag="wq_sb")
                wk_sb = p1w.tile([128, KC * GD], BF16, tag="wk_sb")
                wv_sb = p1w.tile([128, KC * GD], BF16, tag="wv_sb")
                cost_sb = p1big.tile([128, LT], F32, tag="cost_sb")
                sint_sb = p1big.tile([128, LT], F32, tag="sint_sb")
                gtab_sb = p1big.tile([128, 4 * HG], F32, tag="gtab_sb")
                bvrow_sb = p1big.tile([1, GD], F32, tag="bvrow_sb")
                bvb = p1big.tile([128, GD], F32, tag="bvb")
                for c in range(KC):
                    nc.sync.dma_start(out=xT_sb[:, c * LT:(c + 1) * LT],
                                      in_=xT[c, :, :])
                for c in range(KC):
                    nc.sync.dma_start(out=wk_sb[:, c * GD:(c + 1) * GD],
                                      in_=wk[c, :, :])
                    nc.sync.dma_start(out=wv_sb[:, c * GD:(c + 1) * GD],
                                      in_=wv[c, :, :])
                    nc.sync.dma_start(out=wq_sb[:, c * GD:(c + 1) * GD],
                                      in_=wq[c, :, :])
                nc.sync.dma_start(out=cost_sb[:, :], in_=cost[:, :])
                nc.sync.dma_start(out=sint_sb[:, :], in_=sint[:, :])
                nc.sync.dma_start(out=gtab_sb[:, :], in_=gtab[:, :])
                nc.sync.dma_start(out=bvrow_sb[0:1, :], in_=bvrow[:, :])
                nc.gpsimd.partition_broadcast(bvb[:, :], bvrow_sb[0:1, :])

                def qk_proj(tn, w_sb, brow, grow, ssq_row, rot):
                    """Project + rope (pre-norm) + ssq partials.

                    d-outer / t-pair / c-inner: each lhsT chunk loads once
                    per token pair (2 matmuls per LDWEIGHTS).
                    """
                    ssq_ps = [ssqps.tile([1, BLK], F32, tag=f"ssq{t}",
                                         name=f"ssq_{tn}{t}")
                              for t in range(NFR)]
                    for d in range(HG):
                        for tp in range(2):
                            ts = (2 * tp, 2 * tp + 1)
                            pst = {t: p1ps.tile([128, BLK], F32,
                                                tag=f"pp{t % 2}",
                                                name=f"pp{t % 2}")
                                   for t in ts}
                            for c in range(KC):
                                for t in ts:
                                    nc.tensor.matmul(
                                        pst[t][:, :],
                                        w_sb[:, c * GD + d * 128:
                                             c * GD + (d + 1) * 128],
                                        xT_sb[:, c * LT + t * BLK:
                                              c * LT + (t + 1) * BLK],
                                        start=(c == 0), stop=(c == KC - 1))
                            for t in ts:
                                ps = pst[t]
                                ur = p1sb.tile([128, BLK], F32, tag="ur",
                                               name="ur")
                                nc.vector.tensor_scalar_add(
                                    ur[:, :], ps[:, :],
                                    btab_sb[:, brow + d:brow + d + 1])
                                sq = p1sb.tile([128, BLK], BF16, tag="sqsb",
                                               name="sqsb")
                                nc.vector.tensor_tensor(
                                    sq[:, :], ur[:, :], ur[:, :],
                                    mybir.AluOpType.mult)
                                nc.tensor.matmul(
                                    ssq_ps[t][:, :], ones_bf[:, :], sq[:, :],
                                    start=(d == 0), stop=(d == HG - 1))
                                # rope (pre-normalization)
                                usw = p1sb.tile([128, BLK], F32, tag="usw",
                                                name="usw")
                                nc.scalar.dma_start(out=usw[0:64, :],
                                                    in_=ur[64:128, :])
                                nc.scalar.dma_start(out=usw[64:128, :],
                                                    in_=ur[0:64, :])
                                sl = slice(t * BLK, (t + 1) * BLK)
                                t1 = p1sb.tile([128, BLK], BF16,
                                               tag="rope_t1", name="rope_t1")
                                t2 = p1sb.tile([128, BLK], BF16,
                                               tag="rope_t2", name="rope_t2")
                                nc.vector.scalar_tensor_tensor(
                                    t1[:, :], ur[:, :],
                                    gtab_sb[:, grow + d:grow + d + 1],
                                    cost_sb[:, sl], mybir.AluOpType.mult,
                                    mybir.AluOpType.mult)
                                nc.vector.scalar_tensor_tensor(
                                    t2[:, :], usw[:, :],
                                    gtab_sb[:, grow + HG + d:
                                            grow + HG + d + 1],
                                    sint_sb[:, sl], mybir.AluOpType.mult,
                                    mybir.AluOpType.mult)
                                nc.vector.tensor_tensor(
                                    rot[:, d * LT + t * BLK:
                                        d * LT + (t + 1) * BLK],
                                    t1[:, :], t2[:, :], mybir.AluOpType.add)
                    # ship mask-zeroed ssq partials to both half slots
                    for t in range(NFR):
                        sc_sb = sqcp.tile([1, BLK], F32, tag="ssqc",
                                          name="ssqc")
                        nc.vector.tensor_copy(sc_sb[:, :], ssq_ps[t][:, :])
                        for half in range(2):
                            mrow = sqcp.tile([1, BLK], F32, tag="mrow",
                                             name="mrow")
                            nc.vector.tensor_scalar_mul(
                                mrow[:, :], sc_sb[:, :],
                                selm_sb[0:1, half:half + 1])
                            nc.scalar.dma_start(
                                out=ssq_in.ap()[ssq_row, half,
                                                t * BLK:(t + 1) * BLK],
                                in_=mrow[0:1, :])

                # ---- k proj+rope -> stage unnormalized k -> k AllGather ----
                qk_proj("k", wk_sb, HG, 2 * HG, 1, krot)
                for d in range(HG):
                    for t in range(NFR):
                        nc.sync.dma_start(
                            out=kT_view[d * 128:(d + 1) * 128,
                                        t * BLK:(t + 1) * BLK],
                            in_=krot[:, d * LT + t * BLK:
                                     d * LT + (t + 1) * BLK])
                for d in range(HG):
                    nc.scalar.dma_start(
                        out=kT_view[d * 128:(d + 1) * 128, LT:SLOT],
                        in_=zero_bf[:, 0:SLOT - LT])
                nc.gpsimd.collective_compute(
                    "AllGather", mybir.AluOpType.bypass,
                    ins=[k_in.ap().opt()], outs=[k_out.ap().opt()],
                    replica_groups=PAIRS)

                # ---- q proj+rope (overlaps the k AllGather) -> AllReduce ----
                qk_proj("q", wq_sb, 0, 0, 0, qrot)
                nc.gpsimd.collective_compute(
                    "AllReduce", mybir.AluOpType.add,
                    ins=[ssq_in.ap().opt()], outs=[ssq_all.ap().opt()],
                    replica_groups=ALL8)

                # ---- v projection (t13-pairs outer, c inner) -> v AG ----
                nc.scalar.dma_start(out=v_view[LT:SLOT, :],
                                    in_=zero_bf[0:SLOT - LT, :])
                for vp in range(7):
                    ts = [t for t in (2 * vp, 2 * vp + 1) if t < 13]
                    vps = {t: p1ps.tile([128, BLK], F32, tag=f"pp{t % 2}",
                                        name=f"vp{t % 2}") for t in ts}
                    for c in range(KC):
                        for t in ts:
                            tsz = 128 if t < 12 else LT - 12 * 128
                            nc.tensor.matmul(
                                vps[t][0:tsz, 0:GD],
                                xT_sb[:, c * LT + t * 128:
                                      c * LT + t * 128 + tsz],
                                wv_sb[:, c * GD:(c + 1) * GD],
                                start=(c == 0), stop=(c == KC - 1))
                    for t in ts:
                        tsz = 128 if t < 12 else LT - 12 * 128
                        vsb = p1sb.tile([128, GD], BF16, tag="vsb",
                                        name="vsb")
                        nc.vector.tensor_tensor(vsb[0:tsz, :],
                                                vps[t][0:tsz, 0:GD],
                                                bvb[0:tsz, :],
                                                mybir.AluOpType.add)
                        nc.sync.dma_start(
                            out=v_view[t * 128:t * 128 + tsz, :],
                            in_=vsb[0:tsz, :])
                nc.gpsimd.collective_compute(
                    "AllGather", mybir.AluOpType.bypass,
                    ins=[v_in.ap().opt()], outs=[v_out.ap().opt()],
                    replica_groups=PAIRS)

                # ---- rstd tables from the AllReduce ----
                # q: blend halves with my role mask, broadcast, scale qrot
                sa_q = p1big.tile([2, LT], F32, tag="sa_q")
                nc.sync.dma_start(out=sa_q[:, :], in_=ssq_all[0, :, :])
                rst_q = p1big.tile([2, LT], F32, tag="rst_q")
                nc.scalar.activation(rst_q[:, :], sa_q[:, :],
                                     mybir.ActivationFunctionType.Sqrt,
                                     bias=eps_sb[:, :], scale=1.0 / D)
                nc.vector.reciprocal_approx_fast(sa_q[:, :], rst_q[:, :])
                rowb = p1big.tile([1, LT], F32, tag="rowb")
                nc.scalar.dma_start(out=rowb[0:1, :], in_=sa_q[1:2, :])
                rowt = p1big.tile([1, LT], F32, tag="rowt")
                nc.vector.tensor_scalar_mul(rowt[:, :], sa_q[0:1, :],
                                            selm_sb[0:1, 0:1])
                rowf = p1big.tile([1, LT], F32, tag="rowf")
                nc.vector.scalar_tensor_tensor(
                    rowf[:, :], rowb[:, :], selm_sb[0:1, 1:2], rowt[:, :],
                    mybir.AluOpType.mult, mybir.AluOpType.add)
                bcq = p1big.tile([128, LT], F32, tag="bcq")
                nc.gpsimd.partition_broadcast(bcq[:, :], rowf[:, :])
                for d in range(HG):
                    for t in range(NFR):
                        sl = slice(d * LT + t * BLK, d * LT + (t + 1) * BLK)
                        nc.vector.tensor_tensor(
                            qT_sb[:, sl], qrot[:, sl],
                            bcq[:, t * BLK:(t + 1) * BLK],
                            mybir.AluOpType.mult)

                # k: canonical (no blend) -> padded bf16 stream row -> bcast
                sa_k = p1big.tile([2, LT], F32, tag="sa_q", name="sa_k")
                nc.sync.dma_start(out=sa_k[:, :], in_=ssq_all[1, :, :])
                rst_k = p1big.tile([2, LT], F32, tag="rst_q", name="rst_k")
                nc.scalar.activation(rst_k[:, :], sa_k[:, :],
                                     mybir.ActivationFunctionType.Sqrt,
                                     bias=eps_sb[:, :], scale=1.0 / D)
                nc.vector.reciprocal_approx_fast(sa_k[:, :], rst_k[:, :])
                rowk1 = p1big.tile([1, LT], F32, tag="rowb", name="rowk1")
                nc.scalar.dma_start(out=rowk1[0:1, :], in_=sa_k[1:2, :])
                rk_pad = p1big.tile([1, KS], BF16, tag="rk_pad")
                nc.vector.memset(rk_pad[:, :], 0.0)
                nc.vector.tensor_copy(rk_pad[0:1, 0:LT], sa_k[0:1, :])
                nc.vector.tensor_copy(rk_pad[0:1, SLOT:SLOT + LT],
                                      rowk1[0:1, :])
                nc.gpsimd.partition_broadcast(rk_bc[:, :], rk_pad[0:1, :])

            # ===== Phase 2: attention (3 local heads) =====
            with tc.tile_pool(name="p4w", bufs=1) as p4w:
                wo_sb = p4w.tile([128, KC * D], BF16, tag="wo_sb")
                for c in range(KC):
                    nc.sync.dma_start(out=wo_sb[:, c * D:(c + 1) * D],
                                      in_=wo[c, :, :])

                with tc.tile_pool(name="a_k", bufs=2) as akp, \
                     tc.tile_pool(name="a_v", bufs=2) as avp, \
                     tc.tile_pool(name="a_p", bufs=6) as app, \
                     tc.tile_pool(name="a_sb", bufs=4) as asb, \
                     tc.tile_pool(name="a_ps", bufs=2, space="PSUM") as aps, \
                     tc.tile_pool(name="acc_ps", bufs=2, space="PSUM") as accps, \
                     tc.tile_pool(name="sum_ps", bufs=2, space="PSUM") as sumps:

                    for h in range(HG):
                        ktr = akp.tile([128, KS], BF16, tag="ktr", name="ktr")
                        ktn = akp.tile([128, KS], BF16, tag="ktn", name="ktn")
                        vt_sb = avp.tile([128, NKT, 128], BF16, tag="vt_sb",
                                         name="vt_sb")
                        for s in range(2):
                            nc.sync.dma_start(
                                out=ktr[:, s * SLOT:(s + 1) * SLOT],
                                in_=k_out.ap()[s, :]
                                .rearrange("(d t) -> d t", t=SLOT)
                                [h * 128:(h + 1) * 128, :])
                            nc.scalar.dma_start(
                                out=vt_sb[:, s * 13:(s + 1) * 13, :],
                                in_=v_out.ap()[s, :]
                                .rearrange("(t c) -> t c", c=GD)
                                [:, h * 128:(h + 1) * 128]
                                .rearrange("(tt p) c -> p tt c", p=128))
                        # consumer-side k normalization (canonical rstd)
                        nc.vector.tensor_tensor(ktn[:, :], ktr[:, :],
                                                rk_bc[:, :],
                                                mybir.AluOpType.mult)

                        for f in range(NFR):
                            q_ap = qT_sb[:, h * LT + f * BLK:
                                         h * LT + (f + 1) * BLK]
                            acc = accps.tile([128, BLK], F32, tag="acc",
                                             name="acc")
                            sums = sumps.tile([1, BLK], F32, tag="sums",
                                              name="sums")
                            # batches: pairs of same-bias tiles
                            tl = GROUPS[f]
                            batches = []
                            pend = []
                            for kt, bci in tl:
                                if pend and not (pend[0][1] is None
                                                 and bci is None):
                                    batches.append(pend)
                                    pend = []
                                pend.append((kt, bci))
                                if len(pend) == 2 or bci is not None:
                                    batches.append(pend)
                                    pend = []
                            if pend:
                                batches.append(pend)
                            n_tiles = len(tl)
                            ti = 0
                            for batch in batches:
                                nb = len(batch)
                                sc = aps.tile([128, 2, 512], F32, tag="sc",
                                              name="sc")
                                pr = app.tile([128, 2, BLK], BF16, tag="pr",
                                              name="pr")
                                for j, (kt, _) in enumerate(batch):
                                    nc.tensor.matmul(
                                        sc[:, j, 0:BLK],
                                        ktn[:, kt * 128:(kt + 1) * 128],
                                        q_ap, start=True, stop=True)
                                bci = batch[0][1]
                                bias = (0.0 if bci is None
                                        else biasc_sb[:, bci:bci + 1])
                                nc.scalar.activation(
                                    pr[:, 0:nb, :], sc[:, 0:nb, 0:BLK],
                                    mybir.ActivationFunctionType.Exp,
                                    bias=bias, scale=SCALE)
                                for j, (kt, _) in enumerate(batch):
                                    nc.tensor.matmul(
                                        acc[:, :], vt_sb[:, kt, :],
                                        pr[:, j, :],
                                        start=(ti + j == 0),
                                        stop=(ti + j == n_tiles - 1))
                                for j, (kt, _) in enumerate(batch):
                                    nc.tensor.matmul(
                                        sums[:, :], ones_bf[:, :],
                                        pr[:, j, :],
                                        start=(ti + j == 0),
                                        stop=(ti + j == n_tiles - 1))
                                ti += nb
                            rec = asb.tile([1, BLK], F32, tag="rec",
                                           name="rec")
                            nc.vector.reciprocal_approx_fast(rec[:, :],
                                                             sums[:, :])
                            recb = asb.tile([128, BLK], F32, tag="recb",
                                            name="recb")
                            nc.gpsimd.partition_broadcast(recb[:, :],
                                                          rec[:, :])
                            ab = asb.tile([128, BLK], BF16, tag="ab",
                                          name="ab")
                            nc.vector.tensor_tensor(ab[:, :], acc[:, :],
                                                    recb[:, :],
                                                    mybir.AluOpType.mult)
                            nc.scalar.dma_start(
                                out=a2a_in[h].ap()[2 * f, :, :],
                                in_=ab[:, :])
                            nc.scalar.dma_start(
                                out=a2a_in[h].ap()[2 * f + 1, :, :],
                                in_=ab[:, :])
                        nc.gpsimd.collective_compute(
                            "AllToAll", mybir.AluOpType.bypass,
                            ins=[a2a_in[h].ap().opt()],
                            outs=[a2a_out[h].ap().opt()],
                            replica_groups=ALL8)

                    # ---- blend A2A slots into attn_all ----
                    for h in range(HG):
                        for g in range(4):
                            raw = asb.tile([128, 2, BLK], BF16, tag="raw",
                                           name="raw")
                            nc.sync.dma_start(
                                out=raw[:, :, :],
                                in_=a2a_out[h].ap()[2 * g:2 * g + 2, :, :]
                                .rearrange("s p b -> p s b"))
                            tmpb = asb.tile([128, BLK], BF16, tag="tmpb",
                                            name="tmpb")
                            nc.vector.tensor_scalar_mul(
                                tmpb[:, :], raw[:, 0, :], selm_sb[:, 0:1])
                            nc.vector.scalar_tensor_tensor(
                                attn_all[:, 3 * g + h, :], raw[:, 1, :],
                                selm_sb[:, 1:2], tmpb[:, :],
                                mybir.AluOpType.mult, mybir.AluOpType.add)

                # ===== Phase 3: o-projection (out^T form, my 390 tokens) ====
                with tc.tile_pool(name="p5sb", bufs=3) as p5sb, \
                     tc.tile_pool(name="p5ps", bufs=3, space="PSUM") as p5ps:
                    for dch in range(KC):
                        ps = p5ps.tile([128, BLK], F32, tag="o_ps",
                                       name="o_ps")
                        for c in range(KC):
                            nc.tensor.matmul(
                                ps[:, :],
                                wo_sb[:, c * D + dch * 128:
                                      c * D + (dch + 1) * 128],
                                attn_all[:, c, :],
                                start=(c == 0), stop=(c == KC - 1))
                        osb = p5sb.tile([128, BLK], F32, tag="osb",
                                        name="osb")
                        nc.vector.tensor_scalar_add(osb[:, :], ps[:, :],
                                                    botab_sb[:, dch:dch + 1])
                        nc.sync.dma_start(
                            out=out_part[dch * 128:(dch + 1) * 128, :],
                            in_=osb[:, :])

    nc.compile()
    return nc


_NC_CACHE = {}


def _get_nc(key=()):
    if key not in _NC_CACHE:
        _NC_CACHE[key] = build_kernel()
    return _NC_CACHE[key]


def _prep_inputs(x, freqs_cos, freqs_sin, Wq, bq, Wk, bk, Wv, bv, Wo, bo,
                 gq, gk, frame_seqlen, debug=False):
    assert int(frame_seqlen) == L
    bf16 = ml_dtypes.bfloat16
    x2d = np.asarray(x, np.float32).reshape(T, D)
    xT_full = np.ascontiguousarray(x2d.T)

    # rope pair permutation (even dims then odd dims within each head)
    perm = np.concatenate([
        np.concatenate([np.arange(0, 128, 2), np.arange(1, 128, 2)]) + 128 * h
        for h in range(H)])
    Wqp = np.asarray(Wq, np.float32)[:, perm]
    Wkp = np.asarray(Wk, np.float32)[:, perm]
    bqp = np.asarray(bq, np.float32)[perm]
    bkp = np.asarray(bk, np.float32)[perm]
    gqp = np.asarray(gq, np.float32)[perm]
    gkp = np.asarray(gk, np.float32)[perm]
    Wv_ = np.asarray(Wv, np.float32)
    Wo_ = np.asarray(Wo, np.float32)
    bv_ = np.asarray(bv, np.float32)
    bo_ = np.asarray(bo, np.float32)

    cosT = np.asarray(freqs_cos, np.float32).T  # [64, T]
    sinT = np.asarray(freqs_sin, np.float32).T
    costab = np.concatenate([cosT, cosT], 0)     # [128, T]
    sintab = np.concatenate([-sinT, sinT], 0)

    wo_tiled = np.ascontiguousarray(Wo_.reshape(KC, 128, D)).astype(bf16)
    botab = np.ascontiguousarray(bo_.reshape(KC, 128).T)

    in_maps = []
    for c in range(NC):
        g, role = c // 2, c % 2
        tok = np.concatenate([np.arange(f * L + role * BLK,
                                        f * L + role * BLK + BLK)
                              for f in range(NFR)])
        gsl = slice(g * GD, (g + 1) * GD)

        def lhsT_chunks(w):  # [D, GD] -> [KC, 128, GD]
            return np.ascontiguousarray(w.reshape(KC, 128, GD)).astype(bf16)

        def swap_half(col):  # [128] -> halves swapped
            return np.concatenate([col[64:], col[:64]])

        gq_cols = [gqp[gsl][d * 128:(d + 1) * 128] for d in range(HG)]
        gk_cols = [gkp[gsl][d * 128:(d + 1) * 128] for d in range(HG)]
        gtab = np.stack(
            gq_cols + [swap_half(cc) for cc in gq_cols] +
            gk_cols + [swap_half(cc) for cc in gk_cols], axis=1)
        btab = np.stack(
            [bqp[gsl][d * 128:(d + 1) * 128] for d in range(HG)] +
            [bkp[gsl][d * 128:(d + 1) * 128] for d in range(HG)], axis=1)
        selm = np.zeros((128, 2), np.float32)
        selm[:, 0] = 1.0 - role
        selm[:, 1] = role

        in_maps.append({
            "xT": np.ascontiguousarray(
                xT_full[:, tok].reshape(KC, 128, LT)).astype(bf16),
            "wq": lhsT_chunks(Wqp[:, gsl]),
            "wk": lhsT_chunks(Wkp[:, gsl]),
            "wv": lhsT_chunks(Wv_[:, gsl]),
            "wo": wo_tiled,
            "cost": np.ascontiguousarray(costab[:, tok]),
            "sint": np.ascontiguousarray(sintab[:, tok]),
            "selm": selm,
            "biasc": BIASC,
            "gtab": np.ascontiguousarray(gtab.astype(np.float32)),
            "btab": np.ascontiguousarray(btab.astype(np.float32)),
            "bvrow": np.ascontiguousarray(bv_[gsl][None, :]),
            "botab": botab,
        })
    return (), in_maps


def kernel(x, freqs_cos, freqs_sin, Wq, bq, Wk, bk, Wv, bv, Wo, bo,
           gq, gk, frame_seqlen):
    key, in_maps = _prep_inputs(x, freqs_cos, freqs_sin, Wq, bq, Wk, bk,
                                Wv, bv, Wo, bo, gq, gk, frame_seqlen)
    nc = _get_nc(key)
    res = run_bass_kernel_spmd(nc, in_maps, core_ids=list(range(NC)))
    out = np.empty((1, T, D), np.float32)
    for c in range(NC):
        out[0, c * BLK:(c + 1) * BLK, :] = res.results[c]["out_part"].T
    return out
